# revision 2
# baseline (speedup 1.0000x reference)
# Bass/Tile TRN2 kernel for nn_Conv1D_style: out = ((x * (cluster@style_L)) @ weight) * (cluster@style_R)
#
# Sharding: data-parallel over the batch dim. Each of the 8 cores gets a
# 1024-row slice of x/cluster and a full (replicated) weight/style_L/style_R.
#
# Per-core plan (M=1024 batch, K=4096 din, N=4096 dout), all matmuls bf16
# with fp32 PSUM accumulation:
#   aT[k] = xT[k] * (style_L[:, kslice].T @ clusterT)  -> bf16, SBUF-resident.
#   y[m,n] = sum_k aT[k][:, mslice].T @ W[k, nslice]   (32 accumulating MMs)
#   out[m,n] = y[m,n] * (clusterT[:, mslice].T @ style_R[:, nslice])
#
# The aT production is fused with the first n-block's accumulation: n=0,
# m=0..5 accumulate k-outer across 6 PSUM banks so the PE never drains in
# the prologue, and the per-granule PE work (2k x [pair + 6 MMs] ~ 3.1us)
# exceeds the granule DMA time so the xT stream stays hidden. tmpLT for
# each k is a row-packed pair (tile_position upper/lower 64 rows) split
# into two single-bank PSUM halves so 6 accumulators + 2 tmpL banks fit
# the 8-bank PSUM exactly; the aT multiply is split into matching halves
# so m0..3 MMs can start as soon as half A lands.
#
# DMA layout: two HWDGE queues. Sync (SP) carries clT + style_L (4 chunks,
# so the first tmpLT pair only gates on a 256 KiB chunk) + W granules +
# style_R; Activation (Scalar) carries ONLY the xT granules during the
# prologue (so no copy/wait ever delays an xg dma_start issue) and the
# output tiles during the body. tmpR PSUM->SBUF staging is pinned to
# Vector/GpSimd, never Scalar. All sizes tuned so the first matmul issues
# ~10us after kernel start and the PE streams gap-free to the end.

import numpy as np
import ml_dtypes

B, DIN, DOUT, NCL = 8192, 4096, 4096, 64
NCORES = 8
MB = B // NCORES          # batch rows per core
P = 128
NT = 512                  # n tile (dout cols per matmul)
KT = DIN // P             # 32 k tiles
MT = MB // P              # 8 m tiles
NTS = DOUT // NT          # 8 n tiles
FUSED = 6                 # m tiles of n=0 accumulated during the aT prologue
XG = 2                    # k tiles per xT DMA granule (512 KiB: small first gate)
WG = 4                    # k tiles per W DMA granule (n=0 only)
SLC = 8                   # k tiles per style_L DMA chunk

_CACHE = {}
LAST = {}                 # exposes the most recent BassKernelResults for test harnesses


def _build_program():
    import concourse.bacc as bacc
    import concourse.mybir as mybir
    import concourse.tile as tile

    bf16 = mybir.dt.bfloat16
    f32 = mybir.dt.float32

    nc = bacc.Bacc(None, target_bir_lowering=False, debug=False)

    # xT: [granule, partition, k-in-granule, batch]; W: [n, partition, k, nt]
    # cluster/styles arrive duplicated: rows 64-127 = rows 0-63 (row packing).
    xT_d = nc.declare_dram_parameter("xT", [KT // XG, P, XG, MB], bf16, isOutput=False)
    clT_d = nc.declare_dram_parameter("clusterT", [P, MB], bf16, isOutput=False)
    w_d = nc.declare_dram_parameter("weight", [NTS, P, KT, NT], bf16, isOutput=False)
    sL_d = nc.declare_dram_parameter("style_L", [P, DIN], bf16, isOutput=False)
    sR_d = nc.declare_dram_parameter("style_R", [P, DOUT], bf16, isOutput=False)
    out_d = nc.declare_dram_parameter("out", [MB, DOUT], f32, isOutput=True)

    H = NCL  # 64: row-pack halves

    with tile.TileContext(nc) as tc:
        with (
            tc.tile_pool(name="const", bufs=1) as const_pool,
            tc.tile_pool(name="atp", bufs=1) as at_pool,
            tc.tile_pool(name="wp", bufs=2) as w_pool,
            tc.tile_pool(name="xp", bufs=3) as x_pool,
            tc.tile_pool(name="evp", bufs=3) as ev_pool,
            # PSUM budget (8 banks): py 6 x [128,512] accumulators + pl
            # 2 x [128,512] tmpLT halves = 8.
            tc.tile_pool(name="pyp", bufs=6, space="PSUM") as py_pool,
            tc.tile_pool(name="plp", bufs=2, space="PSUM") as pl_pool,
        ):
            # ---- constants. Sync starts serving ~3us earlier than the
            # Activation queue, so everything that gates the first matmuls
            # (clT, the first style_L chunk, the first W granule) goes there,
            # interleaved smallest-first. style_L is chunked so the k=0
            # tmpLT pair only waits on 256 KiB.
            clT = const_pool.tile([P, MB], bf16, name="clT")
            sL = const_pool.tile([P, DIN], bf16, name="sL")
            sR = const_pool.tile([P, DOUT], bf16, name="sR")
            w0 = w_pool.tile([P, KT, NT], bf16, name="w0", tag="wbig")

            KC = SLC * P  # style_L chunk width in din cols
            nc.sync.dma_start(clT[:], clT_d[:])
            nc.sync.dma_start(sL[:, 0:KC], sL_d[:, 0:KC])
            nc.sync.dma_start(w0[:, 0:WG, :], w_d[0, :, 0:WG, :])
            for c in range(1, KT // SLC):
                nc.sync.dma_start(sL[:, c * KC:(c + 1) * KC], sL_d[:, c * KC:(c + 1) * KC])
            nc.sync.dma_start(w0[:, WG:2 * WG, :], w_d[0, :, WG:2 * WG, :])
            nc.sync.dma_start(sR[:], sR_d[:])
            for j in range(2, KT // WG):
                nc.sync.dma_start(
                    w0[:, j * WG:(j + 1) * WG, :],
                    w_d[0, :, j * WG:(j + 1) * WG, :],
                )

            def tmpr_pair(n, m, psum_src="py"):
                """Row-packed pair: tmpR tiles for (m, m+1) at n, staged to SBUF.

                psum_src="pl" borrows the two pl-pool banks instead of two py
                slots — required in the fused prologue where all six py slots
                are held by the open accumulators (a py allocation there
                would deadlock against its own epilogue). The PSUM->SBUF
                staging copies are pinned to Vector/GpSimd so they can never
                sit in front of an xg dma_start in the Scalar queue.
                """
                pool = pl_pool if psum_src == "pl" else py_pool
                tag = "pl" if psum_src == "pl" else "py"
                pra = pool.tile([P, NT], f32, name=f"pr{n}_{m}", tag=tag)
                prb = pool.tile([P, NT], f32, name=f"pr{n}_{m + 1}", tag=tag)
                nc.tensor.matmul(
                    pra[:],
                    clT[:H, m * P:(m + 1) * P],
                    sR[:H, n * NT:(n + 1) * NT],
                    start=True, stop=True, tile_position=(0, 0),
                )
                nc.tensor.matmul(
                    prb[:],
                    clT[H:, (m + 1) * P:(m + 2) * P],
                    sR[H:, n * NT:(n + 1) * NT],
                    start=True, stop=True, tile_position=(H, 0),
                )
                tra = ev_pool.tile([P, NT], f32, name=f"tr{n}_{m}", tag="tr", bufs=6)
                trb = ev_pool.tile([P, NT], f32, name=f"tr{n}_{m + 1}", tag="tr", bufs=6)
                nc.vector.tensor_copy(out=tra[:], in_=pra[:])
                nc.vector.tensor_copy(out=trb[:], in_=prb[:])
                return tra, trb

            def epilogue(n, m, py, tr, split=False):
                if split:
                    # tail only: halve the multiply and fan the output DMA
                    # across both queues so the last bytes leave ASAP.
                    o0 = ev_pool.tile([P, NT // 2], f32, name=f"ot{n}_{m}a", tag="ot")
                    o1 = ev_pool.tile([P, NT // 2], f32, name=f"ot{n}_{m}b", tag="ot")
                    nc.vector.tensor_mul(out=o0[:], in0=py[:, 0:NT // 2], in1=tr[:, 0:NT // 2])
                    nc.scalar.dma_start(
                        out_d[m * P:(m + 1) * P, n * NT:n * NT + NT // 2], o0[:])
                    nc.vector.tensor_mul(out=o1[:], in0=py[:, NT // 2:], in1=tr[:, NT // 2:])
                    nc.sync.dma_start(
                        out_d[m * P:(m + 1) * P, n * NT + NT // 2:(n + 1) * NT], o1[:])
                    return
                ot = ev_pool.tile([P, NT], f32, name=f"ot{n}_{m}", tag="ot")
                nc.vector.tensor_mul(out=ot[:], in0=py[:], in1=tr[:])
                nc.scalar.dma_start(
                    out_d[m * P:(m + 1) * P, n * NT:(n + 1) * NT], ot[:]
                )

            # ---- fused prologue: aT production + n0/m0..5 k-outer accumulation ----
            py_f = [
                py_pool.tile([P, NT], f32, name=f"py0_{m}", tag="py")
                for m in range(FUSED)
            ]
            at_tiles = []
            tr_f = []
            for g in range(KT // XG):
                xg = x_pool.tile([P, XG, MB], bf16, name=f"xg{g}", tag="xg")
                nc.scalar.dma_start(xg[:], xT_d[g])
                for j in range(XG):
                    k = g * XG + j
                    # tmpLT: row-packed pair, one single-bank PSUM half each
                    pla = pl_pool.tile([P, NT], f32, name=f"pla{k}", tag="pl")
                    plb = pl_pool.tile([P, NT], f32, name=f"plb{k}", tag="pl")
                    nc.tensor.matmul(
                        pla[:],
                        sL[:H, k * P:(k + 1) * P],
                        clT[:H, 0:NT],
                        start=True, stop=True, tile_position=(0, 0),
                    )
                    nc.tensor.matmul(
                        plb[:],
                        sL[H:, k * P:(k + 1) * P],
                        clT[H:, NT:MB],
                        start=True, stop=True, tile_position=(H, 0),
                    )
                    at_k = at_pool.tile([P, MB], bf16, name=f"at{k}", tag=f"at{k}")
                    nc.vector.tensor_mul(
                        out=at_k[:, 0:NT], in0=xg[:, j, 0:NT], in1=pla[:])
                    nc.vector.tensor_mul(
                        out=at_k[:, NT:MB], in0=xg[:, j, NT:MB], in1=plb[:])
                    at_tiles.append(at_k)
                    for m in range(FUSED):
                        nc.tensor.matmul(
                            py_f[m][:],
                            at_k[:, m * P:(m + 1) * P],
                            w0[:, k, :],
                            start=(k == 0), stop=(k == KT - 1),
                        )
                if g == 3:
                    # tmpR for the fused m tiles; placed here (PE has slack in
                    # the prologue) so it doesn't gate the kernel start on sR
                    tr_f += tmpr_pair(0, 0, psum_src="pl")
                elif g == 5:
                    tr_f += tmpr_pair(0, 2, psum_src="pl")
                elif g == 7:
                    tr_f += tmpr_pair(0, 4, psum_src="pl")
            for m in range(FUSED):
                epilogue(0, m, py_f[m], tr_f[m])

            # ---- standard m-pair body: two 32-MM groups with the packed tmpR
            # pair injected mid-group (the deep MM pipeline hides its
            # LDWEIGHTS; at a group boundary it costs a full extra slot) ----
            def body_pair(n, m, wn, tail=False):
                tra = trb = None
                for mm in (m, m + 1):
                    py = py_pool.tile([P, NT], f32, name=f"py{n}_{mm}", tag="py")
                    for k in range(KT):
                        nc.tensor.matmul(
                            py[:],
                            at_tiles[k][:, mm * P:(mm + 1) * P],
                            wn[:, k, :],
                            start=(k == 0), stop=(k == KT - 1),
                        )
                        if mm == m and k == KT // 2:
                            tra, trb = tmpr_pair(n, m)
                    epilogue(n, mm, py, tra if mm == m else trb, split=tail)

            # rest of n=0
            for m in range(FUSED, MT, 2):
                body_pair(0, m, w0)
            # n = 1..7
            for n in range(1, NTS):
                wn = w_pool.tile([P, KT, NT], bf16, name=f"w{n}", tag="wbig")
                nc.sync.dma_start(wn[:], w_d[n])
                for m in range(0, MT, 2):
                    body_pair(n, m, wn, tail=(n == NTS - 1 and m == MT - 2))

    nc.finalize()
    return nc


def _get_program():
    if "nc" not in _CACHE:
        _CACHE["nc"] = _build_program()
    return _CACHE["nc"]


def kernel(x, cluster, weight, style_L, style_R):
    import os

    # The NTFF trace path needs an antenv hook this container lacks; never
    # let a stray BASS_TRACE env take the run down that path.
    os.environ.setdefault("BASS_NEVER_TRACE", "1")
    from concourse.bass_utils import run_bass_kernel_spmd

    nc = _get_program()
    bf16 = ml_dtypes.bfloat16

    # W: [din, dout] -> [n, p, k, nt] partition-major for contiguous DMA
    w_bf = np.asarray(weight, dtype=np.float32).astype(bf16)
    w_r = np.ascontiguousarray(
        w_bf.reshape(KT, P, NTS, NT).transpose(2, 1, 0, 3)
    )
    # styles/cluster duplicated across both 64-row halves for row packing
    sL1 = np.asarray(style_L, dtype=np.float32).astype(bf16)
    sR1 = np.asarray(style_R, dtype=np.float32).astype(bf16)
    sL = np.ascontiguousarray(np.vstack([sL1, sL1]))
    sR = np.ascontiguousarray(np.vstack([sR1, sR1]))

    in_maps = []
    for c in range(NCORES):
        xs = np.asarray(x[c * MB:(c + 1) * MB], dtype=np.float32)
        xT = np.ascontiguousarray(xs.T).astype(bf16)          # [DIN, MB]
        # [din, mb] -> [granule, p, k-in-granule, mb]
        xT_r = np.ascontiguousarray(
            xT.reshape(KT // XG, XG, P, MB).transpose(0, 2, 1, 3)
        )
        clT1 = np.ascontiguousarray(
            np.asarray(cluster[c * MB:(c + 1) * MB], dtype=np.float32).T
        ).astype(bf16)
        clT = np.ascontiguousarray(np.vstack([clT1, clT1]))
        in_maps.append(
            {"xT": xT_r, "clusterT": clT, "weight": w_r, "style_L": sL, "style_R": sR}
        )

    res = run_bass_kernel_spmd(nc, in_maps, list(range(NCORES)))
    LAST["results"] = res
    LAST["in_maps"] = in_maps
    out = np.concatenate(
        [np.asarray(res.results[c]["out"], dtype=np.float32) for c in range(NCORES)],
        axis=0,
    )
    return out


# revision 4
# speedup vs baseline: 1.0378x; 1.0378x over previous
# Bass/Tile TRN2 kernel for nn_Conv1D_style: out = ((x * (cluster@style_L)) @ weight) * (cluster@style_R)
#
# Sharding: data-parallel over the batch dim. Each of the 8 cores gets a
# 1024-row slice of x/cluster and a full (replicated) weight/style_L/style_R.
#
# Per-core plan (M=1024 batch, K=4096 din, N=4096 dout), all matmuls bf16
# with fp32 PSUM accumulation:
#   aT[k] = xT[k] * (style_L[:, kslice].T @ clusterT)  -> bf16, SBUF-resident.
#   y[m,n] = sum_k aT[k][:, mslice].T @ W[k, nslice]   (32 accumulating MMs)
#   out[m,n] = y[m,n] * (clusterT[:, mslice].T @ style_R[:, nslice])
#
# The aT production is fused with the first n-block's accumulation (n=0,
# m=0..3 accumulate k-outer across 4 PSUM banks). The tmpLT pairs are
# SOFTWARE-PIPELINED one k ahead of the fused MMs: the PE order per k is
# [pair_{k+1}; fused_k x4], so the Vector at-multiply of k (which the
# fused_k LDWEIGHTS waits on) runs under the previous cycle's MMs instead
# of serially in front of them. The prologue is Vector-paced at ~1.22us/k
# (one [128,1024] psum-operand multiply per k); pl double-buffering gives
# the pair->mul->pair chain a ~0.8us margin.
#
# PSUM (8 banks): py 4 x [128,512] accumulators + pl 2 x [128,1024]
# tmpLT tiles (2 banks each). In the body the pl pool is otherwise idle,
# so the per-group tmpR pair borrows it — its banks were released by
# copies many groups back, which kills the just-in-time bank waits the
# py pool showed at every injection.
#
# DMA: two HWDGE queues. Sync (SP) carries clT + style_L (4 chunks, so
# the first tmpLT pair only gates on 256 KiB) + W granules + style_R;
# Activation (Scalar) carries ONLY the xT granules during the prologue
# (no copy ever delays an xg dma_start issue: prologue tr copies go to
# Scalar only via ACTIVATE after their granule's dma is issued) and the
# output tiles during the body. tmpR staging splits Vector/Scalar.

import numpy as np
import ml_dtypes

B, DIN, DOUT, NCL = 8192, 4096, 4096, 64
NCORES = 8
MB = B // NCORES          # batch rows per core
P = 128
NT = 512                  # n tile (dout cols per matmul)
KT = DIN // P             # 32 k tiles
MT = MB // P              # 8 m tiles
NTS = DOUT // NT          # 8 n tiles
FUSED = 4                 # m tiles of n=0 accumulated during the aT prologue
XG = 2                    # k tiles per xT DMA granule (512 KiB: small first gate)
WG = 4                    # k tiles per W DMA granule (n=0 only)
SLC = 8                   # k tiles per style_L DMA chunk

_CACHE = {}
LAST = {}                 # exposes the most recent BassKernelResults for test harnesses


def _build_program():
    import concourse.bacc as bacc
    import concourse.mybir as mybir
    import concourse.tile as tile

    bf16 = mybir.dt.bfloat16
    f32 = mybir.dt.float32

    nc = bacc.Bacc(None, target_bir_lowering=False, debug=False)

    # xT: [granule, partition, k-in-granule, batch]; W: [n, partition, k, nt]
    # cluster/styles arrive duplicated: rows 64-127 = rows 0-63 (row packing).
    xT_d = nc.declare_dram_parameter("xT", [KT // XG, P, XG, MB], bf16, isOutput=False)
    clT_d = nc.declare_dram_parameter("clusterT", [P, MB], bf16, isOutput=False)
    w_d = nc.declare_dram_parameter("weight", [NTS, P, KT, NT], bf16, isOutput=False)
    sL_d = nc.declare_dram_parameter("style_L", [P, DIN], bf16, isOutput=False)
    sR_d = nc.declare_dram_parameter("style_R", [P, DOUT], bf16, isOutput=False)
    out_d = nc.declare_dram_parameter("out", [MB, DOUT], f32, isOutput=True)

    H = NCL  # 64: row-pack halves

    with tile.TileContext(nc) as tc:
        with (
            tc.tile_pool(name="const", bufs=1) as const_pool,
            tc.tile_pool(name="atp", bufs=1) as at_pool,
            tc.tile_pool(name="wp", bufs=2) as w_pool,
            tc.tile_pool(name="xp", bufs=3) as x_pool,
            tc.tile_pool(name="evp", bufs=3) as ev_pool,
            # PSUM budget (8 banks): py 4 x [128,512] accumulators + pl
            # 2 x [128,1024] (2 banks each) = 8.
            tc.tile_pool(name="pyp", bufs=4, space="PSUM") as py_pool,
            tc.tile_pool(name="plp", bufs=2, space="PSUM") as pl_pool,
        ):
            # ---- constants. Sync starts serving ~3us earlier than the
            # Activation queue, so everything that gates the first matmuls
            # (clT, the first style_L chunk, the first W granule) goes there,
            # interleaved smallest-first.
            clT = const_pool.tile([P, MB], bf16, name="clT")
            sL = const_pool.tile([P, DIN], bf16, name="sL")
            sR = const_pool.tile([P, DOUT], bf16, name="sR")
            w0 = w_pool.tile([P, KT, NT], bf16, name="w0", tag="wbig")

            KC = SLC * P  # style_L chunk width in din cols
            nc.sync.dma_start(clT[:], clT_d[:])
            nc.sync.dma_start(sL[:, 0:KC], sL_d[:, 0:KC])
            nc.sync.dma_start(w0[:, 0:WG, :], w_d[0, :, 0:WG, :])
            for c in range(1, KT // SLC):
                nc.sync.dma_start(sL[:, c * KC:(c + 1) * KC], sL_d[:, c * KC:(c + 1) * KC])
            nc.sync.dma_start(w0[:, WG:2 * WG, :], w_d[0, :, WG:2 * WG, :])
            nc.sync.dma_start(sR[:], sR_d[:])
            for j in range(2, KT // WG):
                nc.sync.dma_start(
                    w0[:, j * WG:(j + 1) * WG, :],
                    w_d[0, :, j * WG:(j + 1) * WG, :],
                )

            def tmpr_pair(n, m, copy_eng=("vector", "scalar")):
                """Row-packed pair: tmpR tiles for (m, m+1) at n, staged to SBUF.

                PSUM comes from the pl pool, which the body otherwise leaves
                idle — its banks were freed by copies many groups ago, so the
                pair MMs never wait. One staging copy goes to Vector, one to
                Scalar (ACTIVATE Copy), halving the Vector tail per group.
                """
                prp = pl_pool.tile([P, MB], f32, name=f"prf{n}_{m}", tag="pl")
                pra, prb = prp[:, 0:NT], prp[:, NT:MB]
                nc.tensor.matmul(
                    pra,
                    clT[:H, m * P:(m + 1) * P],
                    sR[:H, n * NT:(n + 1) * NT],
                    start=True, stop=True, tile_position=(0, 0),
                )
                nc.tensor.matmul(
                    prb,
                    clT[H:, (m + 1) * P:(m + 2) * P],
                    sR[H:, n * NT:(n + 1) * NT],
                    start=True, stop=True, tile_position=(H, 0),
                )
                tra = ev_pool.tile([P, NT], f32, name=f"tr{n}_{m}", tag="tr", bufs=6)
                trb = ev_pool.tile([P, NT], f32, name=f"tr{n}_{m + 1}", tag="tr", bufs=6)
                def _copy(eng, dst, src):
                    if eng == "vector":
                        nc.vector.tensor_copy(out=dst, in_=src)
                    else:
                        nc.scalar.copy(out=dst, in_=src)
                _copy(copy_eng[0], tra[:], pra)
                _copy(copy_eng[1], trb[:], prb)
                return tra, trb

            def epilogue(n, m, py, tr, split=False):
                if split:
                    # tail only: halve the multiply and fan the output DMA
                    # across both queues so the last bytes leave ASAP.
                    o0 = ev_pool.tile([P, NT // 2], f32, name=f"ot{n}_{m}a", tag="ota")
                    o1 = ev_pool.tile([P, NT // 2], f32, name=f"ot{n}_{m}b", tag="otb")
                    nc.vector.tensor_mul(out=o0[:], in0=py[:, 0:NT // 2], in1=tr[:, 0:NT // 2])
                    nc.scalar.dma_start(
                        out_d[m * P:(m + 1) * P, n * NT:n * NT + NT // 2], o0[:])
                    nc.vector.tensor_mul(out=o1[:], in0=py[:, NT // 2:], in1=tr[:, NT // 2:])
                    nc.sync.dma_start(
                        out_d[m * P:(m + 1) * P, n * NT + NT // 2:(n + 1) * NT], o1[:])
                    return
                ot = ev_pool.tile([P, NT], f32, name=f"ot{n}_{m}", tag="ot")
                nc.vector.tensor_mul(out=ot[:], in0=py[:], in1=tr[:])
                nc.scalar.dma_start(
                    out_d[m * P:(m + 1) * P, n * NT:(n + 1) * NT], ot[:]
                )

            def lt_pair(k):
                """tmpLT row-packed pair for k into a fresh pl tile."""
                pl = pl_pool.tile([P, MB], f32, name=f"pl{k}", tag="pl")
                nc.tensor.matmul(
                    pl[:, 0:NT],
                    sL[:H, k * P:(k + 1) * P],
                    clT[:H, 0:NT],
                    start=True, stop=True, tile_position=(0, 0),
                )
                nc.tensor.matmul(
                    pl[:, NT:MB],
                    sL[H:, k * P:(k + 1) * P],
                    clT[H:, NT:MB],
                    start=True, stop=True, tile_position=(H, 0),
                )
                return pl

            # ---- fused prologue: aT production + n0/m0..3 k-outer
            # accumulation, tmpLT pairs pipelined one k ahead ----
            py_f = [
                py_pool.tile([P, NT], f32, name=f"py0_{m}", tag="py")
                for m in range(FUSED)
            ]
            at_tiles = []
            tr_f = []
            pl_tiles = {0: lt_pair(0)}
            for g in range(KT // XG):
                xg = x_pool.tile([P, XG, MB], bf16, name=f"xg{g}", tag="xg")
                nc.scalar.dma_start(xg[:], xT_d[g])
                for j in range(XG):
                    k = g * XG + j
                    if k + 1 < KT:
                        pl_tiles[k + 1] = lt_pair(k + 1)
                    at_k = at_pool.tile([P, MB], bf16, name=f"at{k}", tag=f"at{k}")
                    nc.vector.tensor_mul(
                        out=at_k[:], in0=xg[:, j, :], in1=pl_tiles.pop(k)[:])
                    at_tiles.append(at_k)
                    for m in range(FUSED):
                        nc.tensor.matmul(
                            py_f[m][:],
                            at_k[:, m * P:(m + 1) * P],
                            w0[:, k, :],
                            start=(k == 0), stop=(k == KT - 1),
                        )
                if g == 3:
                    # tmpR for the fused m tiles; placed here (the pl slot it
                    # reuses was freed by an at-multiply two k's back) so it
                    # doesn't gate the kernel start on sR. Copies on Scalar
                    # only, after this granule's xg dma is already issued.
                    tr_f += tmpr_pair(0, 0, copy_eng=("scalar", "scalar"))
                elif g == 5:
                    tr_f += tmpr_pair(0, 2, copy_eng=("scalar", "scalar"))
            for m in range(FUSED):
                epilogue(0, m, py_f[m], tr_f[m])

            # ---- standard m-pair body: two 32-MM groups with the packed tmpR
            # pair injected mid-group (the deep MM pipeline hides its
            # LDWEIGHTS; at a group boundary it costs a full extra slot) ----
            def body_pair(n, m, wn, tail=False):
                tra = trb = None
                for mm in (m, m + 1):
                    py = py_pool.tile([P, NT], f32, name=f"py{n}_{mm}", tag="py")
                    for k in range(KT):
                        nc.tensor.matmul(
                            py[:],
                            at_tiles[k][:, mm * P:(mm + 1) * P],
                            wn[:, k, :],
                            start=(k == 0), stop=(k == KT - 1),
                        )
                        if mm == m and k == KT // 2:
                            tra, trb = tmpr_pair(n, m)
                    epilogue(n, mm, py, tra if mm == m else trb, split=tail)

            # rest of n=0
            for m in range(FUSED, MT, 2):
                body_pair(0, m, w0)
            # n = 1..7
            for n in range(1, NTS):
                wn = w_pool.tile([P, KT, NT], bf16, name=f"w{n}", tag="wbig")
                nc.sync.dma_start(wn[:], w_d[n])
                for m in range(0, MT, 2):
                    body_pair(n, m, wn, tail=(n == NTS - 1 and m == MT - 2))

    nc.finalize()
    return nc


def _get_program():
    if "nc" not in _CACHE:
        _CACHE["nc"] = _build_program()
    return _CACHE["nc"]


def kernel(x, cluster, weight, style_L, style_R):
    import os

    # The NTFF trace path needs an antenv hook this container lacks; never
    # let a stray BASS_TRACE env take the run down that path.
    os.environ.setdefault("BASS_NEVER_TRACE", "1")
    from concourse.bass_utils import run_bass_kernel_spmd

    nc = _get_program()
    bf16 = ml_dtypes.bfloat16

    # W: [din, dout] -> [n, p, k, nt] partition-major for contiguous DMA
    w_bf = np.asarray(weight, dtype=np.float32).astype(bf16)
    w_r = np.ascontiguousarray(
        w_bf.reshape(KT, P, NTS, NT).transpose(2, 1, 0, 3)
    )
    # styles/cluster duplicated across both 64-row halves for row packing
    sL1 = np.asarray(style_L, dtype=np.float32).astype(bf16)
    sR1 = np.asarray(style_R, dtype=np.float32).astype(bf16)
    sL = np.ascontiguousarray(np.vstack([sL1, sL1]))
    sR = np.ascontiguousarray(np.vstack([sR1, sR1]))

    in_maps = []
    for c in range(NCORES):
        xs = np.asarray(x[c * MB:(c + 1) * MB], dtype=np.float32)
        xT = np.ascontiguousarray(xs.T).astype(bf16)          # [DIN, MB]
        # [din, mb] -> [granule, p, k-in-granule, mb]
        xT_r = np.ascontiguousarray(
            xT.reshape(KT // XG, XG, P, MB).transpose(0, 2, 1, 3)
        )
        clT1 = np.ascontiguousarray(
            np.asarray(cluster[c * MB:(c + 1) * MB], dtype=np.float32).T
        ).astype(bf16)
        clT = np.ascontiguousarray(np.vstack([clT1, clT1]))
        in_maps.append(
            {"xT": xT_r, "clusterT": clT, "weight": w_r, "style_L": sL, "style_R": sR}
        )

    res = run_bass_kernel_spmd(nc, in_maps, list(range(NCORES)))
    LAST["results"] = res
    LAST["in_maps"] = in_maps
    out = np.concatenate(
        [np.asarray(res.results[c]["out"], dtype=np.float32) for c in range(NCORES)],
        axis=0,
    )
    return out


# revision 5
# speedup vs baseline: 1.0395x; 1.0017x over previous
# Bass/Tile TRN2 kernel for nn_Conv1D_style: out = ((x * (cluster@style_L)) @ weight) * (cluster@style_R)
#
# Sharding: data-parallel over the batch dim. Each of the 8 cores gets a
# 1024-row slice of x/cluster and a full (replicated) weight/style_L/style_R.
#
# Per-core plan (M=1024 batch, K=4096 din, N=4096 dout), all matmuls bf16
# with fp32 PSUM accumulation:
#   aT[k] = xT[k] * (style_L[:, kslice].T @ clusterT)  -> bf16, SBUF-resident.
#   y[m,n] = sum_k aT[k][:, mslice].T @ W[k, nslice]   (32 accumulating MMs)
#   out[m,n] = y[m,n] * (clusterT[:, mslice].T @ style_R[:, nslice])
#
# The aT production is fused with the first n-block's accumulation (n=0,
# m=0..3 accumulate k-outer across 4 PSUM banks). The tmpLT pairs are
# SOFTWARE-PIPELINED one k ahead of the fused MMs, so the Vector
# at-multiply of k (which fused_k's LDWEIGHTS waits on) runs under the
# previous cycle's MMs. The prologue paces at the Vector multiply
# (~1.22us/k: one [128,1024] psum-operand multiply per k); the PE's
# ~0.13us/k of slack absorbs the tmpR pairs for BOTH n=0 and n=1 (their
# PSUM->SBUF staging runs on the otherwise-idle Scalar engine), so the
# first 6 body groups carry no packed-pair injections at all. A 64-row
# LDWEIGHTS cannot be pulled ahead past an in-flight full-width MM (row
# conflict), so every injected pair costs ~0.2us on top of its slot —
# worth paying only where the PE has no slack (n>=2).
#
# xT streams in 1 MiB granules (XG=4) on the Activation queue with a
# 4-deep SBUF buffer: granule issues are then gated by at-multiplies
# ~12 k's back, keeping the issue->ring->complete chain far ahead of
# consumption (2-deep buffering measurably starved the PE ~7us/run).
#
# PSUM (8 banks): py 4 x [128,512] accumulators + pl 2 x [128,1024]
# (2 banks each) shared by tmpLT pairs, prologue tmpR borrows, and the
# body's injected tmpR pairs (the pl banks are long-released by copies
# when each injection lands, unlike the py accumulators).
#
# DMA: Sync (SP) carries clT + style_L (4 chunks, so the first tmpLT
# pair only gates on 256 KiB) + style_R + W granules; Activation
# (Scalar) carries ONLY xT during the prologue and the output tiles
# during the body. Outputs: the final group fans its halves across both
# queues to shorten the tail.

import numpy as np
import ml_dtypes

B, DIN, DOUT, NCL = 8192, 4096, 4096, 64
NCORES = 8
MB = B // NCORES          # batch rows per core
P = 128
NT = 512                  # n tile (dout cols per matmul)
KT = DIN // P             # 32 k tiles
MT = MB // P              # 8 m tiles
NTS = DOUT // NT          # 8 n tiles
FUSED = 4                 # m tiles of n=0 accumulated during the aT prologue
XG = 4                    # k tiles per xT DMA granule
WG = 4                    # k tiles per W DMA granule (n=0 only)
SLC = 8                   # k tiles per style_L DMA chunk

_CACHE = {}
LAST = {}                 # exposes the most recent BassKernelResults for test harnesses


def _build_program():
    import concourse.bacc as bacc
    import concourse.mybir as mybir
    import concourse.tile as tile

    bf16 = mybir.dt.bfloat16
    f32 = mybir.dt.float32

    nc = bacc.Bacc(None, target_bir_lowering=False, debug=False)

    # xT: [granule, partition, k-in-granule, batch]; W: [n, partition, k, nt]
    # cluster/styles arrive duplicated: rows 64-127 = rows 0-63 (row packing).
    xT_d = nc.declare_dram_parameter("xT", [KT // XG, P, XG, MB], bf16, isOutput=False)
    clT_d = nc.declare_dram_parameter("clusterT", [P, MB], bf16, isOutput=False)
    w_d = nc.declare_dram_parameter("weight", [NTS, P, KT, NT], bf16, isOutput=False)
    sL_d = nc.declare_dram_parameter("style_L", [P, DIN], bf16, isOutput=False)
    sR_d = nc.declare_dram_parameter("style_R", [P, DOUT], bf16, isOutput=False)
    out_d = nc.declare_dram_parameter("out", [MB, DOUT], f32, isOutput=True)

    H = NCL  # 64: row-pack halves

    with tile.TileContext(nc) as tc:
        with (
            tc.tile_pool(name="const", bufs=1) as const_pool,
            tc.tile_pool(name="atp", bufs=1) as at_pool,
            tc.tile_pool(name="wp", bufs=2) as w_pool,
            tc.tile_pool(name="xp", bufs=4) as x_pool,
            tc.tile_pool(name="evp", bufs=3) as ev_pool,
            # PSUM budget (8 banks): py 4 x [128,512] accumulators + pl
            # 2 x [128,1024] (2 banks each) = 8.
            tc.tile_pool(name="pyp", bufs=4, space="PSUM") as py_pool,
            tc.tile_pool(name="plp", bufs=2, space="PSUM") as pl_pool,
        ):
            clT = const_pool.tile([P, MB], bf16, name="clT")
            sL = const_pool.tile([P, DIN], bf16, name="sL")
            sR = const_pool.tile([P, DOUT], bf16, name="sR")
            w0 = w_pool.tile([P, KT, NT], bf16, name="w0", tag="wbig")

            KC = SLC * P  # style_L chunk width in din cols
            nc.sync.dma_start(clT[:], clT_d[:])
            nc.sync.dma_start(sL[:, 0:KC], sL_d[:, 0:KC])
            nc.sync.dma_start(w0[:, 0:WG, :], w_d[0, :, 0:WG, :])
            nc.sync.dma_start(sL[:, KC:2 * KC], sL_d[:, KC:2 * KC])
            nc.sync.dma_start(sR[:], sR_d[:])
            for c in range(2, KT // SLC):
                nc.sync.dma_start(sL[:, c * KC:(c + 1) * KC], sL_d[:, c * KC:(c + 1) * KC])
            for j in range(1, KT // WG):
                nc.sync.dma_start(
                    w0[:, j * WG:(j + 1) * WG, :],
                    w_d[0, :, j * WG:(j + 1) * WG, :],
                )

            def tmpr_pair(n, m, copy_eng=("vector", "scalar")):
                """Row-packed pair: tmpR tiles (bf16) for (m, m+1) at n.

                PSUM borrows the pl pool — in the prologue its slot was freed
                by an at-multiply a cycle earlier, in the body the pool is
                otherwise idle, so the pair MMs never wait on a bank.
                """
                prp = pl_pool.tile([P, MB], f32, name=f"prf{n}_{m}", tag="pl")
                pra, prb = prp[:, 0:NT], prp[:, NT:MB]
                nc.tensor.matmul(
                    pra,
                    clT[:H, m * P:(m + 1) * P],
                    sR[:H, n * NT:(n + 1) * NT],
                    start=True, stop=True, tile_position=(0, 0),
                )
                nc.tensor.matmul(
                    prb,
                    clT[H:, (m + 1) * P:(m + 2) * P],
                    sR[H:, n * NT:(n + 1) * NT],
                    start=True, stop=True, tile_position=(H, 0),
                )
                tra = ev_pool.tile([P, NT], bf16, name=f"tr{n}_{m}", tag="tr", bufs=14)
                trb = ev_pool.tile([P, NT], bf16, name=f"tr{n}_{m + 1}", tag="tr", bufs=14)
                for eng, dst, src in ((copy_eng[0], tra, pra), (copy_eng[1], trb, prb)):
                    if eng == "vector":
                        nc.vector.tensor_copy(out=dst[:], in_=src)
                    else:
                        nc.scalar.copy(out=dst[:], in_=src)
                return tra, trb

            def epilogue(n, m, py, tr, split=False):
                if split:
                    # tail only: halve the multiply and fan the output DMA
                    # across both queues so the last bytes leave ASAP.
                    o0 = ev_pool.tile([P, NT // 2], f32, name=f"ot{n}_{m}a", tag="ota")
                    o1 = ev_pool.tile([P, NT // 2], f32, name=f"ot{n}_{m}b", tag="otb")
                    nc.vector.tensor_mul(out=o0[:], in0=py[:, 0:NT // 2], in1=tr[:, 0:NT // 2])
                    nc.scalar.dma_start(
                        out_d[m * P:(m + 1) * P, n * NT:n * NT + NT // 2], o0[:])
                    nc.vector.tensor_mul(out=o1[:], in0=py[:, NT // 2:], in1=tr[:, NT // 2:])
                    nc.sync.dma_start(
                        out_d[m * P:(m + 1) * P, n * NT + NT // 2:(n + 1) * NT], o1[:])
                    return
                ot = ev_pool.tile([P, NT], f32, name=f"ot{n}_{m}", tag="ot")
                nc.vector.tensor_mul(out=ot[:], in0=py[:], in1=tr[:])
                nc.scalar.dma_start(
                    out_d[m * P:(m + 1) * P, n * NT:(n + 1) * NT], ot[:]
                )

            def lt_pair(k):
                """tmpLT row-packed pair for k into a fresh pl tile."""
                pl = pl_pool.tile([P, MB], f32, name=f"pl{k}", tag="pl")
                nc.tensor.matmul(
                    pl[:, 0:NT],
                    sL[:H, k * P:(k + 1) * P],
                    clT[:H, 0:NT],
                    start=True, stop=True, tile_position=(0, 0),
                )
                nc.tensor.matmul(
                    pl[:, NT:MB],
                    sL[H:, k * P:(k + 1) * P],
                    clT[H:, NT:MB],
                    start=True, stop=True, tile_position=(H, 0),
                )
                return pl

            # ---- fused prologue: aT production + n0/m0..3 k-outer
            # accumulation, tmpLT pairs pipelined one k ahead, tmpR pairs
            # for n=0 and n=1 dropped into the PE slack one per granule ----
            py_f = [
                py_pool.tile([P, NT], f32, name=f"py0_{m}", tag="py")
                for m in range(FUSED)
            ]
            at_tiles = []
            trs = {}   # (n, m) -> staged tmpR tile
            pl_tiles = {0: lt_pair(0)}
            r_sched = [(0, 0), (0, 2), (0, 4), (0, 6), (1, 0), (1, 2), (1, 4), (1, 6)]
            for g in range(KT // XG):
                xg = x_pool.tile([P, XG, MB], bf16, name=f"xg{g}", tag="xg")
                nc.scalar.dma_start(xg[:], xT_d[g])
                for j in range(XG):
                    k = g * XG + j
                    if k + 1 < KT:
                        pl_tiles[k + 1] = lt_pair(k + 1)
                    at_k = at_pool.tile([P, MB], bf16, name=f"at{k}", tag=f"at{k}")
                    nc.vector.tensor_mul(
                        out=at_k[:], in0=xg[:, j, :], in1=pl_tiles.pop(k)[:])
                    at_tiles.append(at_k)
                    for m in range(FUSED):
                        nc.tensor.matmul(
                            py_f[m][:],
                            at_k[:, m * P:(m + 1) * P],
                            w0[:, k, :],
                            start=(k == 0), stop=(k == KT - 1),
                        )
                if g >= 1:
                    rn, rm = r_sched[g - 1]
                    trs[(rn, rm)], trs[(rn, rm + 1)] = tmpr_pair(
                        rn, rm, copy_eng=("scalar", "scalar"))
            rn, rm = r_sched[-1]
            trs[(rn, rm)], trs[(rn, rm + 1)] = tmpr_pair(
                rn, rm, copy_eng=("scalar", "scalar"))
            for m in range(FUSED):
                epilogue(0, m, py_f[m], trs.pop((0, m)))

            # ---- standard m-pair body: two 32-MM groups. For n<=1 the tmpR
            # tiles were staged in the prologue; for n>=2 the packed pair is
            # injected mid-group (costs ~0.2us of LDWEIGHTS row-conflict). ----
            def body_pair(n, m, wn, tail=False):
                tra = trb = None
                if (n, m) in trs:
                    tra, trb = trs.pop((n, m)), trs.pop((n, m + 1))
                for mm in (m, m + 1):
                    py = py_pool.tile([P, NT], f32, name=f"py{n}_{mm}", tag="py")
                    for k in range(KT):
                        nc.tensor.matmul(
                            py[:],
                            at_tiles[k][:, mm * P:(mm + 1) * P],
                            wn[:, k, :],
                            start=(k == 0), stop=(k == KT - 1),
                        )
                        if tra is None and mm == m and k == KT // 2:
                            tra, trb = tmpr_pair(n, m)
                    epilogue(n, mm, py, tra if mm == m else trb, split=tail)

            # rest of n=0
            for m in range(FUSED, MT, 2):
                body_pair(0, m, w0)
            # n = 1..7
            for n in range(1, NTS):
                wn = w_pool.tile([P, KT, NT], bf16, name=f"w{n}", tag="wbig")
                nc.sync.dma_start(wn[:], w_d[n])
                for m in range(0, MT, 2):
                    body_pair(n, m, wn, tail=(n == NTS - 1 and m == MT - 2))

    nc.finalize()
    return nc


def _get_program():
    if "nc" not in _CACHE:
        _CACHE["nc"] = _build_program()
    return _CACHE["nc"]


def kernel(x, cluster, weight, style_L, style_R):
    import os

    # The NTFF trace path needs an antenv hook this container lacks; never
    # let a stray BASS_TRACE env take the run down that path.
    os.environ.setdefault("BASS_NEVER_TRACE", "1")
    from concourse.bass_utils import run_bass_kernel_spmd

    nc = _get_program()
    bf16 = ml_dtypes.bfloat16

    # W: [din, dout] -> [n, p, k, nt] partition-major for contiguous DMA
    w_bf = np.asarray(weight, dtype=np.float32).astype(bf16)
    w_r = np.ascontiguousarray(
        w_bf.reshape(KT, P, NTS, NT).transpose(2, 1, 0, 3)
    )
    # styles/cluster duplicated across both 64-row halves for row packing
    sL1 = np.asarray(style_L, dtype=np.float32).astype(bf16)
    sR1 = np.asarray(style_R, dtype=np.float32).astype(bf16)
    sL = np.ascontiguousarray(np.vstack([sL1, sL1]))
    sR = np.ascontiguousarray(np.vstack([sR1, sR1]))

    in_maps = []
    for c in range(NCORES):
        xs = np.asarray(x[c * MB:(c + 1) * MB], dtype=np.float32)
        xT = np.ascontiguousarray(xs.T).astype(bf16)          # [DIN, MB]
        # [din, mb] -> [granule, p, k-in-granule, mb]
        xT_r = np.ascontiguousarray(
            xT.reshape(KT // XG, XG, P, MB).transpose(0, 2, 1, 3)
        )
        clT1 = np.ascontiguousarray(
            np.asarray(cluster[c * MB:(c + 1) * MB], dtype=np.float32).T
        ).astype(bf16)
        clT = np.ascontiguousarray(np.vstack([clT1, clT1]))
        in_maps.append(
            {"xT": xT_r, "clusterT": clT, "weight": w_r, "style_L": sL, "style_R": sR}
        )

    res = run_bass_kernel_spmd(nc, in_maps, list(range(NCORES)))
    LAST["results"] = res
    LAST["in_maps"] = in_maps
    out = np.concatenate(
        [np.asarray(res.results[c]["out"], dtype=np.float32) for c in range(NCORES)],
        axis=0,
    )
    return out


# revision 8
# speedup vs baseline: 1.0594x; 1.0191x over previous
# Bass/Tile TRN2 kernel for nn_Conv1D_style: out = ((x * (cluster@style_L)) @ weight) * (cluster@style_R)
#
# Sharding: data-parallel over the batch dim. Each of the 8 cores gets a
# 1024-row slice of x/cluster and a full (replicated) weight/style_L/style_R.
#
# Per-core plan (M=1024 batch, K=4096 din, N=4096 dout), all matmuls bf16
# with fp32 PSUM accumulation:
#   aT[k] = xT[k] * (style_L[:, kslice].T @ clusterT)  -> bf16, SBUF-resident.
#   y[m,n] = sum_k aT[k][:, mslice].T @ W[k, nslice]   (32 accumulating MMs)
#   out[m,n] = y[m,n] * (clusterT[:, mslice].T @ style_R[:, nslice])
#
# Prologue: aT production fused with the n=0/m=0..3 k-outer accumulation
# (4 PSUM banks). tmpLT pairs are software-pipelined one k ahead of the
# fused MMs so the Vector at-multiply of k runs under the previous
# cycle's MMs; the prologue paces at that multiply (~1.22us/k). The PE's
# ~0.13us/k slack absorbs the tmpR pairs for n=0 AND n=1 (staged via the
# otherwise-idle Scalar engine), so the first 6 body groups carry no
# injected pairs; n>=2 injections cost ~0.2us each (a 64-row LDWEIGHTS
# cannot be pulled ahead past an in-flight full-width MM).
#
# DMA layout is driven by the per-packet cost model: each partition-row
# packet costs ~300ns fixed + size/22.5GBps per engine, so transfers
# want >=8KiB rows and few chunks. clT/style_L/style_R ship as ONE
# packed dram tensor (cst); the first 4KiB-row chunk (clT + style_L
# k0..7) is all the first tmpLT pair gates on. W for n=0 streams in
# k-chunks sized so chunk j lands just before the fused MMs reach it.
# xT streams in 1 MiB granules on the Activation queue, 4-deep buffered.
# While the PE waits for the first granule+W chunk (~4us) it runs dummy
# warm-up MMs so the HAM clock gate stays open. The final group's output
# DMAs are row-split across both queues to cut the descriptor tail.
#
# PSUM (8 banks): py 4 x [128,512] accumulators + pl 2 x [128,1024]
# shared by tmpLT pairs, prologue tmpR borrows, and body injections.

import numpy as np
import ml_dtypes

B, DIN, DOUT, NCL = 8192, 4096, 4096, 64
NCORES = 8
MB = B // NCORES          # batch rows per core
P = 128
NT = 512                  # n tile (dout cols per matmul)
KT = DIN // P             # 32 k tiles
MT = MB // P              # 8 m tiles
NTS = DOUT // NT          # 8 n tiles
FUSED = 4                 # m tiles of n=0 accumulated during the aT prologue
XG = 4                    # k tiles per xT DMA granule
SL0 = MB                  # cst column offsets: [clT | style_L | style_R]
SR0 = MB + DIN
W0CUTS = [0, 4, 12, 20, 28, 32]   # n=0 W chunk boundaries (k tiles)

_CACHE = {}
LAST = {}                 # exposes the most recent BassKernelResults for test harnesses


def _build_program():
    import concourse.bacc as bacc
    import concourse.mybir as mybir
    import concourse.tile as tile

    bf16 = mybir.dt.bfloat16
    f32 = mybir.dt.float32

    nc = bacc.Bacc(None, target_bir_lowering=False, debug=False)

    # xT: [granule, partition, k-in-granule, batch]; W: [n, partition, k, nt]
    # cluster/styles arrive packed in cst and duplicated across both 64-row
    # halves for row packing.
    xT_d = nc.declare_dram_parameter("xT", [KT // XG, P, XG, MB], bf16, isOutput=False)
    cst_d = nc.declare_dram_parameter("cst", [P, MB + DIN + DOUT], bf16, isOutput=False)
    w_d = nc.declare_dram_parameter("weight", [NTS, P, KT, NT], bf16, isOutput=False)
    out_d = nc.declare_dram_parameter("out", [MB, DOUT], f32, isOutput=True)

    H = NCL  # 64: row-pack halves

    with tile.TileContext(nc) as tc:
        with (
            tc.tile_pool(name="const", bufs=1) as const_pool,
            tc.tile_pool(name="atp", bufs=1) as at_pool,
            tc.tile_pool(name="wp", bufs=2) as w_pool,
            tc.tile_pool(name="xp", bufs=4) as x_pool,
            tc.tile_pool(name="evp", bufs=3) as ev_pool,
            # PSUM budget (8 banks): py 4 x [128,512] accumulators + pl
            # 2 x [128,1024] (2 banks each) = 8.
            tc.tile_pool(name="pyp", bufs=4, space="PSUM") as py_pool,
            tc.tile_pool(name="plp", bufs=2, space="PSUM") as pl_pool,
        ):
            cst = const_pool.tile([P, MB + DIN + DOUT], bf16, name="cst")
            w0 = w_pool.tile([P, KT, NT], bf16, name="w0", tag="wbig")

            # Sync queue, in gating order: (clT + sL k0-7) gates the first
            # pair; W0 chunks land just ahead of the fused stream; sL rest
            # and sR slot into the remaining capacity before they're read.
            nc.sync.dma_start(cst[:, 0:2 * MB], cst_d[:, 0:2 * MB])
            nc.sync.dma_start(w0[:, 0:4, :], w_d[0, :, 0:4, :])
            nc.sync.dma_start(w0[:, 4:12, :], w_d[0, :, 4:12, :])
            nc.sync.dma_start(cst[:, 2 * MB:SR0], cst_d[:, 2 * MB:SR0])
            nc.sync.dma_start(w0[:, 12:20, :], w_d[0, :, 12:20, :])
            nc.sync.dma_start(cst[:, SR0:], cst_d[:, SR0:])
            nc.sync.dma_start(w0[:, 20:28, :], w_d[0, :, 20:28, :])
            nc.sync.dma_start(w0[:, 28:32, :], w_d[0, :, 28:32, :])

            def tmpr_pair(n, m, copy_eng=("vector", "scalar")):
                """Row-packed pair: tmpR tiles (bf16) for (m, m+1) at n.

                PSUM borrows the pl pool — in the prologue its slot was freed
                by an at-multiply a cycle earlier, in the body the pool is
                otherwise idle, so the pair MMs never wait on a bank.
                """
                prp = pl_pool.tile([P, MB], f32, name=f"prf{n}_{m}", tag="pl")
                pra, prb = prp[:, 0:NT], prp[:, NT:MB]
                nc.tensor.matmul(
                    pra,
                    cst[:H, m * P:(m + 1) * P],
                    cst[:H, SR0 + n * NT:SR0 + (n + 1) * NT],
                    start=True, stop=True, tile_position=(0, 0),
                )
                nc.tensor.matmul(
                    prb,
                    cst[H:, (m + 1) * P:(m + 2) * P],
                    cst[H:, SR0 + n * NT:SR0 + (n + 1) * NT],
                    start=True, stop=True, tile_position=(H, 0),
                )
                tra = ev_pool.tile([P, NT], bf16, name=f"tr{n}_{m}", tag="tr", bufs=14)
                trb = ev_pool.tile([P, NT], bf16, name=f"tr{n}_{m + 1}", tag="tr", bufs=14)
                for eng, dst, src in ((copy_eng[0], tra, pra), (copy_eng[1], trb, prb)):
                    if eng == "vector":
                        nc.vector.tensor_copy(out=dst[:], in_=src)
                    else:
                        nc.scalar.copy(out=dst[:], in_=src)
                return tra, trb

            def epilogue(n, m, py, tr, split=False):
                ot = ev_pool.tile([P, NT], f32, name=f"ot{n}_{m}", tag="ot")
                nc.vector.tensor_mul(out=ot[:], in0=py[:], in1=tr[:])
                if split:
                    # tail only: fan the output rows across both queues so
                    # the last descriptors generate in parallel.
                    nc.scalar.dma_start(
                        out_d[m * P:m * P + P // 2, n * NT:(n + 1) * NT], ot[0:P // 2, :])
                    nc.sync.dma_start(
                        out_d[m * P + P // 2:(m + 1) * P, n * NT:(n + 1) * NT], ot[P // 2:, :])
                else:
                    nc.scalar.dma_start(
                        out_d[m * P:(m + 1) * P, n * NT:(n + 1) * NT], ot[:]
                    )

            def lt_pair(k):
                """tmpLT row-packed pair for k into a fresh pl tile."""
                pl = pl_pool.tile([P, MB], f32, name=f"pl{k}", tag="pl")
                nc.tensor.matmul(
                    pl[:, 0:NT],
                    cst[:H, SL0 + k * P:SL0 + (k + 1) * P],
                    cst[:H, 0:NT],
                    start=True, stop=True, tile_position=(0, 0),
                )
                nc.tensor.matmul(
                    pl[:, NT:MB],
                    cst[H:, SL0 + k * P:SL0 + (k + 1) * P],
                    cst[H:, NT:MB],
                    start=True, stop=True, tile_position=(H, 0),
                )
                return pl

            # ---- fused prologue ----
            py_f = [
                py_pool.tile([P, NT], f32, name=f"py0_{m}", tag="py")
                for m in range(FUSED)
            ]
            at_tiles = []
            trs = {}   # (n, m) -> staged tmpR tile
            pl_tiles = {0: lt_pair(0)}
            # Keep the PE (and HAM) warm between the first pairs (~12.6us)
            # and the first fused MM (~16.5us): dummy MMs into py_f[0],
            # overwritten by the real k=0 start=True matmul.
            for wu in range(14):
                nc.tensor.matmul(
                    py_f[0][:],
                    cst[:, 0:P],
                    cst[:, 0:NT],
                    start=True, stop=True,
                )
            # tmpR staging order: n=0 pairs first (consumed at prologue end),
            # then n=1. All run late in the prologue (sR lands ~37us).
            r_sched = {(4, 1): (0, 0), (4, 3): (0, 2), (5, 1): (0, 4), (5, 3): (0, 6),
                       (6, 1): (1, 0), (6, 3): (1, 2), (7, 1): (1, 4), (7, 3): (1, 6)}
            for g in range(KT // XG):
                xg = x_pool.tile([P, XG, MB], bf16, name=f"xg{g}", tag="xg")
                nc.scalar.dma_start(xg[:], xT_d[g])
                for j in range(XG):
                    k = g * XG + j
                    if k + 1 < KT:
                        pl_tiles[k + 1] = lt_pair(k + 1)
                    at_k = at_pool.tile([P, MB], bf16, name=f"at{k}", tag=f"at{k}")
                    nc.vector.tensor_mul(
                        out=at_k[:], in0=xg[:, j, :], in1=pl_tiles.pop(k)[:])
                    at_tiles.append(at_k)
                    for m in range(FUSED):
                        nc.tensor.matmul(
                            py_f[m][:],
                            at_k[:, m * P:(m + 1) * P],
                            w0[:, k, :],
                            start=(k == 0), stop=(k == KT - 1),
                        )
                    if (g, j) in r_sched:
                        rn, rm = r_sched[(g, j)]
                        trs[(rn, rm)], trs[(rn, rm + 1)] = tmpr_pair(
                            rn, rm, copy_eng=("scalar", "scalar"))
            for m in range(FUSED):
                epilogue(0, m, py_f[m], trs.pop((0, m)))

            # ---- standard m-pair body ----
            def body_pair(n, m, wn, tail=False):
                tra = trb = None
                if (n, m) in trs:
                    tra, trb = trs.pop((n, m)), trs.pop((n, m + 1))
                for mm in (m, m + 1):
                    py = py_pool.tile([P, NT], f32, name=f"py{n}_{mm}", tag="py")
                    for k in range(KT):
                        nc.tensor.matmul(
                            py[:],
                            at_tiles[k][:, mm * P:(mm + 1) * P],
                            wn[:, k, :],
                            start=(k == 0), stop=(k == KT - 1),
                        )
                        if tra is None and mm == m and k == KT // 2:
                            tra, trb = tmpr_pair(n, m)
                    epilogue(n, mm, py, tra if mm == m else trb, split=tail)

            # rest of n=0
            for m in range(FUSED, MT, 2):
                body_pair(0, m, w0)
            # n = 1..7
            for n in range(1, NTS):
                wn = w_pool.tile([P, KT, NT], bf16, name=f"w{n}", tag="wbig")
                nc.sync.dma_start(wn[:], w_d[n])
                for m in range(0, MT, 2):
                    body_pair(n, m, wn, tail=(n == NTS - 1 and m == MT - 2))

    nc.finalize()
    return nc


def _get_program():
    if "nc" not in _CACHE:
        _CACHE["nc"] = _build_program()
    return _CACHE["nc"]


def kernel(x, cluster, weight, style_L, style_R):
    import os

    # The NTFF trace path needs an antenv hook this container lacks; never
    # let a stray BASS_TRACE env take the run down that path.
    os.environ.setdefault("BASS_NEVER_TRACE", "1")
    from concourse.bass_utils import run_bass_kernel_spmd

    nc = _get_program()
    bf16 = ml_dtypes.bfloat16

    # W: [din, dout] -> [n, p, k, nt] partition-major for contiguous DMA
    w_bf = np.asarray(weight, dtype=np.float32).astype(bf16)
    w_r = np.ascontiguousarray(
        w_bf.reshape(KT, P, NTS, NT).transpose(2, 1, 0, 3)
    )
    # styles/cluster duplicated across both 64-row halves for row packing
    sL1 = np.asarray(style_L, dtype=np.float32).astype(bf16)
    sR1 = np.asarray(style_R, dtype=np.float32).astype(bf16)
    sL = np.ascontiguousarray(np.vstack([sL1, sL1]))
    sR = np.ascontiguousarray(np.vstack([sR1, sR1]))

    in_maps = []
    for c in range(NCORES):
        xs = np.asarray(x[c * MB:(c + 1) * MB], dtype=np.float32)
        xT = np.ascontiguousarray(xs.T).astype(bf16)          # [DIN, MB]
        # [din, mb] -> [granule, p, k-in-granule, mb]
        xT_r = np.ascontiguousarray(
            xT.reshape(KT // XG, XG, P, MB).transpose(0, 2, 1, 3)
        )
        clT1 = np.ascontiguousarray(
            np.asarray(cluster[c * MB:(c + 1) * MB], dtype=np.float32).T
        ).astype(bf16)
        clT = np.ascontiguousarray(np.vstack([clT1, clT1]))
        cst = np.ascontiguousarray(np.concatenate([clT, sL, sR], axis=1))
        in_maps.append({"xT": xT_r, "cst": cst, "weight": w_r})

    res = run_bass_kernel_spmd(nc, in_maps, list(range(NCORES)))
    LAST["results"] = res
    LAST["in_maps"] = in_maps
    out = np.concatenate(
        [np.asarray(res.results[c]["out"], dtype=np.float32) for c in range(NCORES)],
        axis=0,
    )
    return out


# revision 12
# speedup vs baseline: 1.0608x; 1.0013x over previous
# Bass/Tile TRN2 kernel for nn_Conv1D_style: out = ((x * (cluster@style_L)) @ weight) * (cluster@style_R)
#
# Sharding: data-parallel over the batch dim. Each of the 8 cores gets a
# 1024-row slice of x/cluster and a full (replicated) weight/style_L/style_R.
#
# Per-core plan (M=1024 batch, K=4096 din, N=4096 dout), all matmuls bf16
# with fp32 PSUM accumulation:
#   aT[k] = xT[k] * (style_L[:, kslice].T @ clusterT)  -> bf16, SBUF-resident.
#   y[m,n] = sum_k aT[k][:, mslice].T @ W[k, nslice]   (32 accumulating MMs)
#   out[m,n] = y[m,n] * (clusterT[:, mslice].T @ style_R[:, nslice])
#
# Prologue: aT production fused with the n=0/m=0..3 k-outer accumulation
# (4 PSUM banks). tmpLT pairs are software-pipelined one k ahead of the
# fused MMs so the Vector at-multiply of k runs under the previous
# cycle's MMs; the prologue paces at that multiply (~1.22us/k). The PE's
# ~0.13us/k slack absorbs the tmpR pairs for n=0 AND n=1 (staged via the
# otherwise-idle Scalar engine), so the first 6 body groups carry no
# injected pairs; n>=2 injections cost ~0.2us each (a 64-row LDWEIGHTS
# cannot be pulled ahead past an in-flight full-width MM).
#
# DMA layout is driven by the per-packet cost model: each partition-row
# packet costs ~300ns fixed + size/22.5GBps per engine, so transfers
# want >=8KiB rows and few chunks. clT/style_L/style_R ship as ONE
# packed dram tensor (cst); the first 4KiB-row chunk (clT + style_L
# k0..7) is all the first tmpLT pair gates on. W for n=0 streams in
# k-chunks sized so chunk j lands just before the fused MMs reach it.
# xT streams in 1 MiB granules on the Activation queue, 4-deep buffered.
# While the PE waits for the first granule+W chunk (~4us) it runs dummy
# warm-up MMs so the HAM clock gate stays open. The final group's output
# DMAs are row-split across both queues to cut the descriptor tail.
#
# PSUM (8 banks): py 4 x [128,512] accumulators + pl 2 x [128,1024]
# shared by tmpLT pairs, prologue tmpR borrows, and body injections.

import numpy as np
import ml_dtypes

B, DIN, DOUT, NCL = 8192, 4096, 4096, 64
NCORES = 8
MB = B // NCORES          # batch rows per core
P = 128
NT = 512                  # n tile (dout cols per matmul)
KT = DIN // P             # 32 k tiles
MT = MB // P              # 8 m tiles
NTS = DOUT // NT          # 8 n tiles
FUSED = 4                 # m tiles of n=0 accumulated during the aT prologue
XG = 4                    # k tiles per xT DMA granule
SL0 = MB                  # cst column offsets: [clT | style_L | style_R]
SR0 = MB + DIN
W0CUTS = [0, 4, 12, 20, 28, 32]   # n=0 W chunk boundaries (k tiles)

_CACHE = {}
LAST = {}                 # exposes the most recent BassKernelResults for test harnesses


def _build_program():
    import concourse.bacc as bacc
    import concourse.mybir as mybir
    import concourse.tile as tile

    bf16 = mybir.dt.bfloat16
    f32 = mybir.dt.float32

    nc = bacc.Bacc(None, target_bir_lowering=False, debug=False)

    # xT: [granule, partition, k-in-granule, batch]; W: [n, partition, k, nt]
    # cluster/styles arrive packed in cst and duplicated across both 64-row
    # halves for row packing.
    xT_d = nc.declare_dram_parameter("xT", [KT // XG, P, XG, MB], bf16, isOutput=False)
    cst_d = nc.declare_dram_parameter("cst", [P, MB + DIN + DOUT], bf16, isOutput=False)
    w_d = nc.declare_dram_parameter("weight", [NTS, P, KT, NT], bf16, isOutput=False)
    out_d = nc.declare_dram_parameter("out", [MB, DOUT], f32, isOutput=True)

    H = NCL  # 64: row-pack halves

    with tile.TileContext(nc) as tc:
        with (
            tc.tile_pool(name="const", bufs=1) as const_pool,
            tc.tile_pool(name="atp", bufs=1) as at_pool,
            tc.tile_pool(name="wp", bufs=2) as w_pool,
            tc.tile_pool(name="xp", bufs=4) as x_pool,
            tc.tile_pool(name="evp", bufs=3) as ev_pool,
            # PSUM budget (8 banks): py 4 x [128,512] accumulators + pl
            # 2 x [128,1024] (2 banks each) = 8.
            tc.tile_pool(name="pyp", bufs=4, space="PSUM") as py_pool,
            tc.tile_pool(name="plp", bufs=2, space="PSUM") as pl_pool,
        ):
            cst = const_pool.tile([P, MB + DIN + DOUT], bf16, name="cst")
            w0 = w_pool.tile([P, KT, NT], bf16, name="w0", tag="wbig")

            # Sync queue, in gating order: (clT + sL k0-7) gates the first
            # pair; W0 chunks land just ahead of the fused stream; sL rest
            # and sR slot into the remaining capacity before they're read.
            nc.sync.dma_start(cst[:, 0:2 * MB], cst_d[:, 0:2 * MB])
            nc.sync.dma_start(w0[:, 0:4, :], w_d[0, :, 0:4, :])
            nc.sync.dma_start(w0[:, 4:12, :], w_d[0, :, 4:12, :])
            nc.sync.dma_start(cst[:, 2 * MB:SR0], cst_d[:, 2 * MB:SR0])
            nc.sync.dma_start(w0[:, 12:20, :], w_d[0, :, 12:20, :])
            nc.sync.dma_start(cst[:, SR0:], cst_d[:, SR0:])
            nc.sync.dma_start(w0[:, 20:28, :], w_d[0, :, 20:28, :])
            nc.sync.dma_start(w0[:, 28:32, :], w_d[0, :, 28:32, :])

            def tmpr_pair(n, m, copy_eng=("vector", "scalar")):
                """Row-packed pair: tmpR tiles (bf16) for (m, m+1) at n.

                PSUM borrows the pl pool — in the prologue its slot was freed
                by an at-multiply a cycle earlier, in the body the pool is
                otherwise idle, so the pair MMs never wait on a bank.
                """
                prp = pl_pool.tile([P, MB], f32, name=f"prf{n}_{m}", tag="pl")
                pra, prb = prp[:, 0:NT], prp[:, NT:MB]
                nc.tensor.matmul(
                    pra,
                    cst[:H, m * P:(m + 1) * P],
                    cst[:H, SR0 + n * NT:SR0 + (n + 1) * NT],
                    start=True, stop=True, tile_position=(0, 0),
                )
                nc.tensor.matmul(
                    prb,
                    cst[H:, (m + 1) * P:(m + 2) * P],
                    cst[H:, SR0 + n * NT:SR0 + (n + 1) * NT],
                    start=True, stop=True, tile_position=(H, 0),
                )
                tra = ev_pool.tile([P, NT], bf16, name=f"tr{n}_{m}", tag="tr", bufs=14)
                trb = ev_pool.tile([P, NT], bf16, name=f"tr{n}_{m + 1}", tag="tr", bufs=14)
                for eng, dst, src in ((copy_eng[0], tra, pra), (copy_eng[1], trb, prb)):
                    if eng == "vector":
                        nc.vector.tensor_copy(out=dst[:], in_=src)
                    else:
                        nc.scalar.copy(out=dst[:], in_=src)
                return tra, trb

            def epilogue(n, m, py, tr, split=False, last=False):
                ot = ev_pool.tile([P, NT], f32, name=f"ot{n}_{m}", tag="ot")
                if last:
                    # very last tile: halve the multiply too, so the first
                    # rows' DMA generation overlaps the second multiply.
                    nc.vector.tensor_mul(
                        out=ot[0:P // 2, :], in0=py[0:P // 2, :], in1=tr[0:P // 2, :])
                    nc.scalar.dma_start(
                        out_d[m * P:m * P + P // 2, n * NT:(n + 1) * NT], ot[0:P // 2, :])
                    nc.vector.tensor_mul(
                        out=ot[P // 2:, :], in0=py[P // 2:, :], in1=tr[P // 2:, :])
                    nc.sync.dma_start(
                        out_d[m * P + P // 2:(m + 1) * P, n * NT:(n + 1) * NT], ot[P // 2:, :])
                    return
                nc.vector.tensor_mul(out=ot[:], in0=py[:], in1=tr[:])
                if split:
                    # tail region: fan the output rows across both queues so
                    # the last descriptors generate in parallel.
                    nc.scalar.dma_start(
                        out_d[m * P:m * P + P // 2, n * NT:(n + 1) * NT], ot[0:P // 2, :])
                    nc.sync.dma_start(
                        out_d[m * P + P // 2:(m + 1) * P, n * NT:(n + 1) * NT], ot[P // 2:, :])
                else:
                    nc.scalar.dma_start(
                        out_d[m * P:(m + 1) * P, n * NT:(n + 1) * NT], ot[:]
                    )

            def lt_pair(k):
                """tmpLT row-packed pair for k into a fresh pl tile."""
                pl = pl_pool.tile([P, MB], f32, name=f"pl{k}", tag="pl")
                nc.tensor.matmul(
                    pl[:, 0:NT],
                    cst[:H, SL0 + k * P:SL0 + (k + 1) * P],
                    cst[:H, 0:NT],
                    start=True, stop=True, tile_position=(0, 0),
                )
                nc.tensor.matmul(
                    pl[:, NT:MB],
                    cst[H:, SL0 + k * P:SL0 + (k + 1) * P],
                    cst[H:, NT:MB],
                    start=True, stop=True, tile_position=(H, 0),
                )
                return pl

            # ---- fused prologue ----
            py_f = [
                py_pool.tile([P, NT], f32, name=f"py0_{m}", tag="py")
                for m in range(FUSED)
            ]
            at_tiles = []
            trs = {}   # (n, m) -> staged tmpR tile
            pl_tiles = {0: lt_pair(0)}
            # Keep the PE (and HAM) warm between the first pairs (~12.6us)
            # and the first fused MM (~16.5us): dummy MMs into py_f[0],
            # overwritten by the real k=0 start=True matmul.
            for wu in range(14):
                nc.tensor.matmul(
                    py_f[0][:],
                    cst[:, 0:P],
                    cst[:, 0:NT],
                    start=True, stop=True,
                )
            # Only the tmpR pairs the prologue epilogues themselves consume
            # are staged here (late: sR lands ~37us). Each borrow flips the
            # pl-pool rotation parity and costs ~0.5us, so body groups
            # self-inject instead — that's cheaper.
            r_sched = {(6, 1): (0, 0), (7, 1): (0, 2)}
            for g in range(KT // XG):
                xg = x_pool.tile([P, XG, MB], bf16, name=f"xg{g}", tag="xg")
                nc.scalar.dma_start(xg[:], xT_d[g])
                for j in range(XG):
                    k = g * XG + j
                    if k + 1 < KT:
                        pl_tiles[k + 1] = lt_pair(k + 1)
                    at_k = at_pool.tile([P, MB], bf16, name=f"at{k}", tag=f"at{k}")
                    nc.vector.tensor_mul(
                        out=at_k[:], in0=xg[:, j, :], in1=pl_tiles.pop(k)[:])
                    at_tiles.append(at_k)
                    for m in range(FUSED):
                        nc.tensor.matmul(
                            py_f[m][:],
                            at_k[:, m * P:(m + 1) * P],
                            w0[:, k, :],
                            start=(k == 0), stop=(k == KT - 1),
                        )
                    if (g, j) in r_sched:
                        rn, rm = r_sched[(g, j)]
                        trs[(rn, rm)], trs[(rn, rm + 1)] = tmpr_pair(
                            rn, rm, copy_eng=("scalar", "scalar"))
            for m in range(FUSED):
                epilogue(0, m, py_f[m], trs.pop((0, m)))

            # ---- standard m-pair body ----
            def body_pair(n, m, wn, tail=False):
                tra = trb = None
                if (n, m) in trs:
                    tra, trb = trs.pop((n, m)), trs.pop((n, m + 1))
                for mm in (m, m + 1):
                    py = py_pool.tile([P, NT], f32, name=f"py{n}_{mm}", tag="py")
                    for k in range(KT):
                        nc.tensor.matmul(
                            py[:],
                            at_tiles[k][:, mm * P:(mm + 1) * P],
                            wn[:, k, :],
                            start=(k == 0), stop=(k == KT - 1),
                        )
                        if tra is None and mm == m and k == KT // 2:
                            tra, trb = tmpr_pair(n, m)
                    epilogue(n, mm, py, tra if mm == m else trb, split=tail,
                             last=tail and mm == m + 1)

            # rest of n=0
            for m in range(FUSED, MT, 2):
                body_pair(0, m, w0)
            # n = 1..7
            for n in range(1, NTS):
                wn = w_pool.tile([P, KT, NT], bf16, name=f"w{n}", tag="wbig")
                nc.sync.dma_start(wn[:], w_d[n])
                for m in range(0, MT, 2):
                    body_pair(n, m, wn, tail=(n == NTS - 1 and m >= MT - 4))

    nc.finalize()
    return nc


def _get_program():
    if "nc" not in _CACHE:
        _CACHE["nc"] = _build_program()
    return _CACHE["nc"]


def kernel(x, cluster, weight, style_L, style_R):
    import os

    # The NTFF trace path needs an antenv hook this container lacks; never
    # let a stray BASS_TRACE env take the run down that path.
    os.environ.setdefault("BASS_NEVER_TRACE", "1")
    from concourse.bass_utils import run_bass_kernel_spmd

    nc = _get_program()
    bf16 = ml_dtypes.bfloat16

    # W: [din, dout] -> [n, p, k, nt] partition-major for contiguous DMA
    w_bf = np.asarray(weight, dtype=np.float32).astype(bf16)
    w_r = np.ascontiguousarray(
        w_bf.reshape(KT, P, NTS, NT).transpose(2, 1, 0, 3)
    )
    # styles/cluster duplicated across both 64-row halves for row packing
    sL1 = np.asarray(style_L, dtype=np.float32).astype(bf16)
    sR1 = np.asarray(style_R, dtype=np.float32).astype(bf16)
    sL = np.ascontiguousarray(np.vstack([sL1, sL1]))
    sR = np.ascontiguousarray(np.vstack([sR1, sR1]))

    in_maps = []
    for c in range(NCORES):
        xs = np.asarray(x[c * MB:(c + 1) * MB], dtype=np.float32)
        xT = np.ascontiguousarray(xs.T).astype(bf16)          # [DIN, MB]
        # [din, mb] -> [granule, p, k-in-granule, mb]
        xT_r = np.ascontiguousarray(
            xT.reshape(KT // XG, XG, P, MB).transpose(0, 2, 1, 3)
        )
        clT1 = np.ascontiguousarray(
            np.asarray(cluster[c * MB:(c + 1) * MB], dtype=np.float32).T
        ).astype(bf16)
        clT = np.ascontiguousarray(np.vstack([clT1, clT1]))
        cst = np.ascontiguousarray(np.concatenate([clT, sL, sR], axis=1))
        in_maps.append({"xT": xT_r, "cst": cst, "weight": w_r})

    res = run_bass_kernel_spmd(nc, in_maps, list(range(NCORES)))
    LAST["results"] = res
    LAST["in_maps"] = in_maps
    out = np.concatenate(
        [np.asarray(res.results[c]["out"], dtype=np.float32) for c in range(NCORES)],
        axis=0,
    )
    return out
